# revision 2
# baseline (speedup 1.0000x reference)
"""DeepSeekMOE grouped masked GEMM kernel for 8 Trainium2 NeuronCores.

Expert-parallel: core g owns expert group g. Per core:
  out_ug = x_ug[g] @ w_ug[g].T   [32, 2816]
  out_dn = x_dn[g] @ w_dn[g].T   [32, 4096]
  rows >= masked_m[g] are zero (applied by zeroing x rows on host).
Output [8, 32, 6912] = concat(out_ug, out_dn) per group.

Memory-bound: ~70 MB of weights stream through each core once. Weights are
transposed on host to [K, N] so SBUF k-slabs load with long contiguous rows.
Matmuls run as float32r (TF32-like, full PE rate at free dim >= 256),
accumulating fp32 in PSUM over the K dimension.
"""
import numpy as np

import concourse.bass as bass
import concourse.bacc as bacc
import concourse.mybir as mybir
import concourse.tile as tile
from concourse.bass_utils import run_bass_kernel_spmd

G, M = 8, 32
K_UG, N_UG = 4096, 2816
K_DN, N_DN = 1408, 4096
N_OUT = N_UG + N_DN
P = 128
KC_UG = K_UG // P  # 32 k-chunks
KC_DN = K_DN // P  # 11 k-chunks
SLAB = 2  # k-chunks per weight DMA

f32 = mybir.dt.float32
f32r = mybir.dt.float32r

TRACE = False  # test.py sets True to capture an NTFF profile
_cache = {}


def _n_chunks(n_total):
    chunks = []
    n0 = 0
    while n0 < n_total:
        nlen = min(512, n_total - n0)
        chunks.append((n0, nlen))
        n0 += nlen
    return chunks


def _slabs(kc_total):
    slabs = []
    c0 = 0
    while c0 < kc_total:
        slen = min(SLAB, kc_total - c0)
        slabs.append((c0, slen))
        c0 += slen
    return slabs


def _build_program(reps=1):
    """reps>1 wraps the body in a HW loop — bench-only, for slope timing."""
    nc = bacc.Bacc("TRN2", target_bir_lowering=False, debug=False)

    xt_ug = nc.dram_tensor("xt_ug", [K_UG, M], f32r, kind="ExternalInput")
    wt_ug = nc.dram_tensor("wt_ug", [K_UG, N_UG], f32r, kind="ExternalInput")
    xt_dn = nc.dram_tensor("xt_dn", [K_DN, M], f32r, kind="ExternalInput")
    wt_dn = nc.dram_tensor("wt_dn", [K_DN, N_DN], f32r, kind="ExternalInput")
    out = nc.dram_tensor("out", [M, N_OUT], f32, kind="ExternalOutput")

    import contextlib

    with contextlib.ExitStack() as stack:
        tc = stack.enter_context(tile.TileContext(nc))
        wpool = stack.enter_context(tc.tile_pool(name="w", bufs=4))
        misc = stack.enter_context(tc.tile_pool(name="misc", bufs=1))
        psum = stack.enter_context(tc.tile_pool(name="psum", bufs=1, space="PSUM"))
        if reps > 1:
            stack.enter_context(tc.For_i(0, reps, 1))
        if True:
            # Stationary activations, loaded once: SBUF [128, KC*M],
            # chunk c at columns [c*M, (c+1)*M).
            xug_t = misc.tile([P, KC_UG * M], f32r, tag="xug")
            nc.sync.dma_start(
                xug_t[:].rearrange("k (c m) -> k c m", c=KC_UG),
                xt_ug[:].rearrange("(c k) m -> k c m", k=P),
            )
            xdn_t = misc.tile([P, KC_DN * M], f32r, tag="xdn")
            nc.sync.dma_start(
                xdn_t[:].rearrange("k (c m) -> k c m", c=KC_DN),
                xt_dn[:].rearrange("(c k) m -> k c m", k=P),
            )

            for wt_d, xt_t, n_tot, kc_tot, out_off, oname in (
                (wt_ug, xug_t, N_UG, KC_UG, 0, "oug"),
                (wt_dn, xdn_t, N_DN, KC_DN, N_UG, "odn"),
            ):
                nch = _n_chunks(n_tot)
                acc = psum.tile([M, n_tot], f32, tag="acc")
                wt_src = wt_d[:].rearrange("(c k) n -> k c n", k=P)
                for c0, slen in _slabs(kc_tot):
                    w_t = wpool.tile([P, slen * n_tot], f32r, tag="w")
                    nc.sync.dma_start(
                        w_t[:].rearrange("k (c n) -> k c n", c=slen),
                        wt_src[:, c0 : c0 + slen, :],
                    )
                    for c in range(slen):
                        kc = c0 + c
                        for n0, nlen in nch:
                            nc.tensor.matmul(
                                acc[:, n0 : n0 + nlen],
                                xt_t[:, bass.ts(kc, M)],
                                w_t[:, c * n_tot + n0 : c * n_tot + n0 + nlen],
                                start=(kc == 0),
                                stop=(kc == kc_tot - 1),
                            )
                o_t = misc.tile([M, n_tot], f32, tag=oname)
                for n0, nlen in nch:
                    nc.vector.tensor_copy(
                        o_t[:, n0 : n0 + nlen], acc[:, n0 : n0 + nlen]
                    )
                nc.sync.dma_start(out[:, out_off : out_off + n_tot], o_t[:])

    nc.compile()
    return nc


def kernel(x_ug, w_ug, x_dn, w_dn, masked_m):
    if "nc" not in _cache:
        _cache["nc"] = _build_program()
    nc = _cache["nc"]

    x_ug = np.asarray(x_ug, dtype=np.float32)
    w_ug = np.asarray(w_ug, dtype=np.float32)
    x_dn = np.asarray(x_dn, dtype=np.float32)
    w_dn = np.asarray(w_dn, dtype=np.float32)
    masked_m = np.asarray(masked_m)

    row = np.arange(M)
    in_maps = []
    for g in range(G):
        valid = (row < int(masked_m[g])).astype(np.float32)[:, None]
        in_maps.append(
            {
                "xt_ug": np.ascontiguousarray((x_ug[g] * valid).T),
                "wt_ug": np.ascontiguousarray(w_ug[g].T),
                "xt_dn": np.ascontiguousarray((x_dn[g] * valid).T),
                "wt_dn": np.ascontiguousarray(w_dn[g].T),
            }
        )

    res = run_bass_kernel_spmd(
        nc, in_maps, core_ids=list(range(G)), trace=TRACE
    )
    if TRACE:
        _cache["last_result"] = res
    return np.stack([res.results[g]["out"] for g in range(G)], axis=0)


# revision 12
# speedup vs baseline: 1.9433x; 1.9433x over previous
"""DeepSeekMOE grouped masked GEMM kernel for 8 Trainium2 NeuronCores.

Expert-parallel: core g owns expert group g. Per core:
  out_ug = x_ug[g] @ w_ug[g].T   [32, 2816]
  out_dn = x_dn[g] @ w_dn[g].T   [32, 4096]
  rows >= masked_m[g] are zero (applied by zeroing x rows on host).
Output [8, 32, 6912] = concat(out_ug, out_dn) per group.

Memory-bound: ~70 MB of weights stream through each core once. Weights are
transposed on host to [K, N] so SBUF k-slabs load with long contiguous rows.
Matmuls run as float32r (TF32-like, full PE rate at free dim >= 256),
accumulating fp32 in PSUM over the K dimension.
"""
import numpy as np

import concourse.bass as bass
import concourse.bacc as bacc
import concourse.mybir as mybir
import concourse.tile as tile
from concourse.bass_utils import run_bass_kernel_spmd

G, M = 8, 32
K_UG, N_UG = 4096, 2816
K_DN, N_DN = 1408, 4096
N_OUT = N_UG + N_DN
P = 128
KC_UG = K_UG // P  # 32 k-chunks
KC_DN = K_DN // P  # 11 k-chunks
SLAB = 2  # k-chunks per weight DMA

f32 = mybir.dt.float32
f32r = mybir.dt.float32r

TRACE = False  # test.py sets True to capture an NTFF profile
_cache = {}


def _n_chunks(n_total):
    chunks = []
    n0 = 0
    while n0 < n_total:
        nlen = min(512, n_total - n0)
        chunks.append((n0, nlen))
        n0 += nlen
    return chunks


def _slabs_of(kc_total, slab):
    slabs = []
    c0 = 0
    while c0 < kc_total:
        slen = min(slab, kc_total - c0)
        slabs.append((c0, slen))
        c0 += slen
    return slabs


def _build_program(reps=1, n_stride=1, dma_frac=1, slab=SLAB, wbufs=4,
                   alt_engine=False, no_dma=False, no_pe=False,
                   io_gpsimd=False):
    """reps>1 wraps the body in a HW loop — bench-only, for slope timing.

    n_stride/dma_frac are bench-only probes: compute every n_stride-th n-chunk
    (halves PE work) / load only 1/dma_frac of each weight slab (halves DMA).
    """
    nc = bacc.Bacc("TRN2", target_bir_lowering=False, debug=False)

    xt_ug = nc.dram_tensor("xt_ug", [K_UG, M], f32r, kind="ExternalInput")
    wt_ug = nc.dram_tensor("wt_ug", [K_UG, N_UG], f32r, kind="ExternalInput")
    xt_dn = nc.dram_tensor("xt_dn", [K_DN, M], f32r, kind="ExternalInput")
    wt_dn = nc.dram_tensor("wt_dn", [K_DN, N_DN], f32r, kind="ExternalInput")
    out = nc.dram_tensor("out", [M, N_OUT], f32, kind="ExternalOutput")

    import contextlib

    with contextlib.ExitStack() as stack:
        tc = stack.enter_context(tile.TileContext(nc))
        wpool = stack.enter_context(tc.tile_pool(name="w", bufs=wbufs))
        misc = stack.enter_context(tc.tile_pool(name="misc", bufs=1))
        psum = stack.enter_context(tc.tile_pool(name="psum", bufs=1, space="PSUM"))
        if reps > 1:
            stack.enter_context(tc.For_i(0, reps, 1))
        if True:
            # Stationary activations, loaded once: SBUF [128, KC*M],
            # chunk c at columns [c*M, (c+1)*M).
            io_eng = nc.gpsimd if io_gpsimd else nc.sync
            xug_t = misc.tile([P, KC_UG * M], f32r, tag="xug")
            io_eng.dma_start(
                xug_t[:].rearrange("k (c m) -> k c m", c=KC_UG),
                xt_ug[:].rearrange("(c k) m -> k c m", k=P),
            )
            xdn_t = misc.tile([P, KC_DN * M], f32r, tag="xdn")
            io_eng.dma_start(
                xdn_t[:].rearrange("k (c m) -> k c m", c=KC_DN),
                xt_dn[:].rearrange("(c k) m -> k c m", k=P),
            )

            for wt_d, xt_t, n_tot, kc_tot, out_off, oname in (
                (wt_ug, xug_t, N_UG, KC_UG, 0, "oug"),
                (wt_dn, xdn_t, N_DN, KC_DN, N_UG, "odn"),
            ):
                nch = _n_chunks(n_tot)
                acc = psum.tile([M, n_tot], f32, tag="acc")
                wt_src = wt_d[:].rearrange("(c k) n -> k c n", k=P)
                for si, (c0, slen) in enumerate(_slabs_of(kc_tot, slab)):
                    w_t = wpool.tile([P, slen * n_tot], f32r, tag="w")
                    eng = nc.scalar if (alt_engine and si % 2) else nc.sync
                    nload = n_tot // dma_frac
                    if not no_dma:
                        eng.dma_start(
                            w_t[:, : slen * nload].rearrange(
                                "k (c n) -> k c n", c=slen
                            ),
                            wt_src[:, c0 : c0 + slen, :nload],
                        )
                    for c in range(slen):
                        kc = c0 + c
                        if no_pe:
                            continue
                        for ni, (n0, nlen) in enumerate(nch):
                            if ni % n_stride:
                                continue
                            nc.tensor.matmul(
                                acc[:, n0 : n0 + nlen],
                                xt_t[:, bass.ts(kc, M)],
                                w_t[:, c * n_tot + n0 : c * n_tot + n0 + nlen],
                                start=(kc == 0),
                                stop=(kc == kc_tot - 1),
                            )
                o_t = misc.tile([M, n_tot], f32, tag=oname)
                for n0, nlen in nch:
                    nc.vector.tensor_copy(
                        o_t[:, n0 : n0 + nlen], acc[:, n0 : n0 + nlen]
                    )
                io_eng.dma_start(out[:, out_off : out_off + n_tot], o_t[:])

    nc.compile()
    return nc


def kernel(x_ug, w_ug, x_dn, w_dn, masked_m):
    if "nc" not in _cache:
        _cache["nc"] = _build_program()
    nc = _cache["nc"]

    x_ug = np.asarray(x_ug, dtype=np.float32)
    w_ug = np.asarray(w_ug, dtype=np.float32)
    x_dn = np.asarray(x_dn, dtype=np.float32)
    w_dn = np.asarray(w_dn, dtype=np.float32)
    masked_m = np.asarray(masked_m)

    row = np.arange(M)
    in_maps = []
    for g in range(G):
        valid = (row < int(masked_m[g])).astype(np.float32)[:, None]
        in_maps.append(
            {
                "xt_ug": np.ascontiguousarray((x_ug[g] * valid).T),
                "wt_ug": np.ascontiguousarray(w_ug[g].T),
                "xt_dn": np.ascontiguousarray((x_dn[g] * valid).T),
                "wt_dn": np.ascontiguousarray(w_dn[g].T),
            }
        )

    res = None
    for attempt in range(3):
        try:
            res = run_bass_kernel_spmd(
                nc, in_maps, core_ids=list(range(G)), trace=TRACE
            )
            break
        except Exception:
            if attempt == 2:
                raise
            # Transient NRT/device failures: reset jax backends and retry.
            import time

            try:
                import jax

                jax.clear_caches()
                import jax.extend.backend as _jb

                _jb.clear_backends()
            except Exception:
                pass
            time.sleep(20.0 * (attempt + 1))
    if TRACE:
        _cache["last_result"] = res
    return np.stack([res.results[g]["out"] for g in range(G)], axis=0)
